# revision 30
# baseline (speedup 1.0000x reference)
"""InterSliceAttention TRN2 kernel (v5).

Reference computation (per batch element b):
    curr = f_curr[b] as [N, C] tokens (N = H*W = 1024, C = 512)
    neigh = [f_prev[b]; f_next[b]] as [2N, C]
    Q = curr @ Wq.T ; K = neigh @ Wk.T ; V = neigh @ Wv.T
    8-head attention (hd = 64), softmax over 2N keys
    out = LayerNorm(curr + attn_out @ Wo.T) * gamma + beta   (LN over C)

Sharding: data-parallel over batch, one element per NeuronCore, weights
replicated. All activations stay channels-first ([C_part, token_free]).

v5 changes over v4 (profile-driven):
  - attnV accumulator PSUM double-buffered (was the per-head 6us PE stall
    that also dropped HAM to 1.2 GHz for the whole attention stream).
  - normalize: evacuate acc PSUM->SBUF first (frees banks), then
    reciprocal_approx_fast (was 7.9us nc.vector.reciprocal).
  - scores in bf16 (1-pass matmul; was f32r 2-pass).
  - K/Q/out projections loop-reordered weight-stationary so LDWEIGHTS
    amortizes over 4/2 moving chunks.
  - LN rsqrt via exp(-0.5*ln(var+eps)) on ACT (Sqrt needed an activation
    table switch); LN apply chain on DVE in bf16 (was 23us of gpsimd).
  - input DMA order prioritizes wk+xn so K-proj starts ~4us earlier.
"""

import numpy as np

NUM_CORES = 8
B, C, H, W = 8, 512, 32, 32
N = H * W          # 1024 query tokens
N2 = 2 * N         # 2048 key tokens
HEADS = 8
HD = C // HEADS    # 64
SCALE = HD ** -0.5
LN_EPS = 1e-5
P = 128
CT = C // P        # 4 channel tiles
JT = N2 // P       # 16 key-token tiles
JP = JT // 2       # 8 key-tile pairs (DoubleRow contracts 256 keys)
FREE = 512
QC = N // FREE     # 2 query chunks
KC = N2 // FREE    # 4 key chunks
VP = 80            # padded per-(sub,head) lhsT stride in fp8 (step%16==0)

# Schraudolph fast-exp, fp8e5 (e5m2) flavor:
# exp(SCALE*x) ~= bitcast_e5m2(int8(A8*x + B8))
A_EXP8 = (1 << 2) / np.log(2.0) * SCALE
B_EXP8 = float((15 << 2) - 366393.0 / (1 << 21))

_CACHE = {}


def _emit(ctx, tc, io, dbg=None):
    import concourse.bass as bass
    from concourse import mybir

    nc = tc.nc
    f32 = mybir.dt.float32
    f32r = mybir.dt.float32r
    i8 = mybir.dt.int8
    bf16 = mybir.dt.bfloat16
    e4 = mybir.dt.float8e4
    e5 = mybir.dt.float8e5
    Alu = mybir.AluOpType
    Act = mybir.ActivationFunctionType
    DR = mybir.MatmulPerfMode.DoubleRow

    def F(ap):  # f32 view of an f32r tile for DVE/ACT consumers
        return ap.bitcast(f32)

    xc_d, w_d, gamma_d, beta_d, y_d = io

    # ---------------- persistent SBUF ----------------
    persist = ctx.enter_context(tc.tile_pool(name="persist", bufs=1))

    ones_col = persist.tile([P, 1], bf16, tag="ones")
    nc.vector.memset(ones_col[:], 1.0)

    xc_sb = [persist.tile([P, N], bf16, tag=f"xc{i}", name=f"xc{i}") for i in range(CT)]
    qt_sb = [persist.tile([P, N], bf16, tag=f"qt{i}", name=f"qt{i}") for i in range(CT)]
    kt_sb = [persist.tile([P, N2], bf16, tag=f"kt{i}", name=f"kt{i}") for i in range(CT)]
    # V packed for DoubleRow: [keys mod 128, key-subtile, head, [V|1] padded]
    vv_sb = [persist.tile([P, 2, HEADS, VP], e4, tag=f"vv{jp}", name=f"vv{jp}")
             for jp in range(JP)]
    aot_sb = [persist.tile([P, N], bf16, tag=f"aot{i}", name=f"aot{i}") for i in range(CT)]
    wo_sb = [persist.tile([P, C], bf16, tag=f"wo{i}", name=f"wo{i}") for i in range(CT)]
    gamma_ct = [persist.tile([P, 1], f32, tag=f"g{i}", name=f"g{i}") for i in range(CT)]
    beta_ct = [persist.tile([P, 1], f32, tag=f"b{i}", name=f"b{i}") for i in range(CT)]
    eps_t = persist.tile([1, 1], f32, tag="eps")
    nc.vector.memset(eps_t[:], LN_EPS)

    # preload activation table set 6 (natural_log_exp_and_others) so exp and
    # ln share one resident table: the auto-placement pass picks per-function
    # first-containing sets (exp->0, ln->5) and thrashes 21 loads otherwise.
    nc.scalar.add_instruction(mybir.InstLoadActFuncSet(
        act_func_set_id=6, name=nc.get_next_instruction_name(),
        ins=[], outs=[]))

    # ---------------- stage A: input DMA + QKV projections ----------------
    # All three projections run as fp8e4 DoubleRow matmuls (the 512-deep
    # contraction folds to 2 passes of 256); inputs arrive pre-interleaved
    # from the host as [pass*128+ki, ko, tokens] with c = pass*256+ko*128+ki.
    with tc.tile_pool(name="stageA", bufs=1) as a_pool, \
         tc.tile_pool(name="psA", bufs=8, space="PSUM") as ps_a:
        xn8_sb = [a_pool.tile([P, 2, N2], e4, tag=f"xn8{p}", name=f"xn8{p}")
                  for p in range(2)]
        xc8_sb = [a_pool.tile([P, 2, N], e4, tag=f"xc8{p}", name=f"xc8{p}")
                  for p in range(2)]
        wq8_sb = [a_pool.tile([P, 2, C], e4, tag=f"wq8{p}", name=f"wq8{p}")
                  for p in range(2)]
        wk8_sb = [a_pool.tile([P, 2, C], e4, tag=f"wk8{p}", name=f"wk8{p}")
                  for p in range(2)]
        wv8_sb = [a_pool.tile([P, 2, C], e4, tag=f"wv8{p}", name=f"wv8{p}")
                  for p in range(2)]

        # DMA spread over the two HWDGE queues; wk8 + xn8 first so K-proj
        # can start as early as possible.
        for p in range(2):
            nc.scalar.dma_start(out=wk8_sb[p][:], in_=w_d["k8"][p * P:(p + 1) * P])
        nc.sync.dma_start(out=xn8_sb[0][:], in_=w_d["xn8"][0:P])
        nc.scalar.dma_start(out=xn8_sb[1][:], in_=w_d["xn8"][P:2 * P])
        for p in range(2):
            nc.sync.dma_start(out=wv8_sb[p][:], in_=w_d["v8"][p * P:(p + 1) * P])
        for p in range(2):
            nc.sync.dma_start(out=xc8_sb[p][:], in_=w_d["xc8"][p * P:(p + 1) * P])
        for p in range(2):
            nc.scalar.dma_start(out=wq8_sb[p][:], in_=w_d["q8"][p * P:(p + 1) * P])
        for i in range(CT):
            nc.sync.dma_start(out=xc_sb[i][:], in_=xc_d[i * P:(i + 1) * P, :])
        for i in range(CT):
            nc.sync.dma_start(out=wo_sb[i][:], in_=w_d["o"][i * P:(i + 1) * P, :])
        for i in range(CT):
            nc.scalar.dma_start(out=gamma_ct[i][:], in_=gamma_d[i * P:(i + 1) * P, :])
            nc.scalar.dma_start(out=beta_ct[i][:], in_=beta_d[i * P:(i + 1) * P, :])

        # K projection: Kt[C,2N] = Wk @ Xn; evacuation alternates ACT / DVE.
        for mo in range(CT):
            pss = [ps_a.tile([P, FREE], f32, tag="mm", name=f"k{mo}_{kc}")
                   for kc in range(KC)]
            for p in range(2):
                for kc in range(KC):
                    nc.tensor.matmul(
                        pss[kc][:],
                        wk8_sb[p][:, :, mo * P:(mo + 1) * P],
                        xn8_sb[p][:, :, kc * FREE:(kc + 1) * FREE],
                        start=(p == 0), stop=(p == 1), perf_mode=DR)
            for kc in range(KC):
                dst = kt_sb[mo][:, kc * FREE:(kc + 1) * FREE]
                if kc % 2 == 0:
                    nc.scalar.copy(out=dst, in_=pss[kc][:])
                else:
                    nc.vector.tensor_copy(out=dst, in_=pss[kc][:])

        # V token-major, packed into vv[jp][:, sub, h, 0:64] (e4m3) + ones col;
        # evacuation on DVE
        for jp in range(JP):
            nc.vector.memset(vv_sb[jp][:, :, :, HD], 1.0)
        for j in range(JT):
            ps = ps_a.tile([P, FREE], f32, tag="mm", name=f"v{j}")
            for p in range(2):
                nc.tensor.matmul(
                    ps[:],
                    xn8_sb[p][:, :, j * P:(j + 1) * P],
                    wv8_sb[p][:],
                    start=(p == 0), stop=(p == 1), perf_mode=DR)
            jp, sub = divmod(j, 2)
            nc.vector.tensor_copy(
                out=vv_sb[jp][:, sub, :, 0:HD],
                in_=ps[:].rearrange("p (h d) -> p h d", h=HEADS))

        # Q projection: Qt[C,N] = Wq @ Xc.
        for mo in range(CT):
            pss = [ps_a.tile([P, FREE], f32, tag="mm", name=f"q{mo}_{qc}")
                   for qc in range(QC)]
            for p in range(2):
                for qc in range(QC):
                    nc.tensor.matmul(
                        pss[qc][:],
                        wq8_sb[p][:, :, mo * P:(mo + 1) * P],
                        xc8_sb[p][:, :, qc * FREE:(qc + 1) * FREE],
                        start=(p == 0), stop=(p == 1), perf_mode=DR)
            for qc in range(QC):
                dst = qt_sb[mo][:, qc * FREE:(qc + 1) * FREE]
                if qc % 2 == 0:
                    nc.scalar.copy(out=dst, in_=pss[qc][:])
                else:
                    nc.vector.tensor_copy(out=dst, in_=pss[qc][:])

    if dbg is not None:
        for i in range(CT):
            nc.sync.dma_start(out=dbg["kt"][i * P:(i + 1) * P, :], in_=kt_sb[i][:])
            nc.sync.dma_start(out=dbg["qt"][i * P:(i + 1) * P, :], in_=qt_sb[i][:])

    # ---------------- stage C: attention ----------------
    # Head-PAIRED stream of rounds r = (hi, jp): the even head's scores MMs
    # run on PE row-groups 0-1 (lhsT base partition 0) and the odd head's on
    # row-groups 2-3 (base partition 64) -- interleaved so the 64-deep PE
    # queue runs them concurrently (the K=64 contraction only fills half the
    # array). exp slabs split 2+2 over ACT (table exp) and DVE (Schraudolph).
    # attnV for round r-1 is emitted mid-round so PE never waits on exp.
    with tc.tile_pool(name="expp", bufs=4) as exp_pool, \
         tc.tile_pool(name="normp", bufs=2) as norm_pool, \
         tc.tile_pool(name="psS", bufs=2, space="PSUM") as sc_pool, \
         tc.tile_pool(name="psAcc", bufs=2, space="PSUM") as acc_pool:
        ROUNDS = CT * JP
        e2_of = {}
        acc_of = {}
        raw_of = {}
        pending = {}

        def attn_half(hi, jp, hp):
            h = 2 * hi + hp
            acc = acc_of[h]
            for qc in range(QC):
                nc.tensor.matmul(
                    acc[:, qc * FREE:(qc + 1) * FREE],
                    vv_sb[jp][:, :, h, 0:HD + 1],
                    e2_of[h][:, :, qc * FREE:(qc + 1) * FREE],
                    start=(jp == 0), stop=(jp == JP - 1),
                    perf_mode=DR)

        def evac_raws(hi):
            # evacuate both accumulators right away (frees the acc PSUM
            # banks); ACT for the even head, DVE for the odd one. Rowsum
            # rows are collected into one [2, N] tile via SBUF-to-SBUF DMA
            # so a single ln+exp pass serves both heads. The reciprocal
            # chains are deferred to spread ACT load.
            # rowsums land at partitions 0 and 32 (engine access must be
            # 32-partition aligned)
            rs2 = norm_pool.tile([33, N], bf16, tag="rs2", name=f"rs{hi}")
            for hp in range(2):
                h = 2 * hi + hp
                raw = norm_pool.tile([HD + 1, N], bf16, tag=f"raw{hp}",
                                     name=f"raw{h}")
                if hp == 0:
                    nc.scalar.copy(out=raw[:], in_=acc_of[h][:])
                else:
                    nc.vector.tensor_copy(out=raw[:], in_=acc_of[h][:])
                raw_of[h] = raw
                nc.sync.dma_start(out=rs2[32 * hp:32 * hp + 1, :],
                                  in_=raw[HD:HD + 1, :])
            return rs2

        def pair_recips(hi, rs2):
            # 1/rowsum = exp(-ln(rowsum)) on ACT: Ln+Exp share a table set;
            # the DVE reciprocal paths are slow (7.9us) or broken. Rows
            # 1..31 hold garbage and are never read.
            lnr = norm_pool.tile([33, N], f32, tag="lnr")
            nc.scalar.activation(lnr[:], rs2[:], Act.Ln)
            recip = norm_pool.tile([33, N], bf16, tag="r", name=f"rcp{hi}")
            nc.scalar.activation(recip[:], lnr[:], Act.Exp, scale=-1.0)
            if dbg is not None and hi == 0:
                rtmp = norm_pool.tile([1, N], f32, tag="rdbg")
                nc.vector.tensor_copy(out=rtmp[:], in_=recip[0:1, :])
                nc.sync.dma_start(out=dbg["st"][3:4, :], in_=rtmp[:])
            return recip

        def finish_head(hi, hp, recip):
            h = 2 * hi + hp
            raw = raw_of[h]
            if hp == 1:
                # partition_broadcast reads partition 0 only: DMA the odd
                # head's recip row down from partition 32 first.
                r1 = norm_pool.tile([1, N], bf16, tag="r1")
                nc.sync.dma_start(out=r1[:], in_=recip[32:33, :])
                rsrc = r1[:]
            else:
                rsrc = recip[0:1, :]
            rb = norm_pool.tile([HD, N], bf16, tag=f"rb{hp}")
            nc.gpsimd.partition_broadcast(rb[:], rsrc)
            # the multiply runs on DVE: mixing op types on gpsimd costs a
            # ~6-7us Q7 program reload per switch (partition_broadcast is
            # the only gpsimd op in the kernel).
            if hp == 0:
                nc.vector.tensor_mul(aot_sb[hi][0:HD, :], raw[0:HD, :], rb[:])
            else:
                # odd heads land at rows 64:128 -> partition-shifting DMA
                ao = norm_pool.tile([HD, N], bf16, tag="ao")
                nc.vector.tensor_mul(ao[:], raw[0:HD, :], rb[:])
                nc.sync.dma_start(out=aot_sb[hi][HD:P, :], in_=ao[:])

        for r in range(ROUNDS):
            hi, jp = divmod(r, JP)
            for task in pending.pop(r, []):
                task()
            if jp == 0:
                for hp in range(2):
                    h = 2 * hi + hp
                    acc_of[h] = acc_pool.tile([HD + 1, N], f32, tag="att",
                                              name=f"acc{h}")
            new_e2 = {}
            for hp in range(2):
                new_e2[hp] = exp_pool.tile([P, 2, N], e5, tag="e2",
                                           name=f"e2_{r}_{hp}")
            for sub in range(2):
                j = 2 * jp + sub
                ss = {}
                for hp in range(2):
                    ss[hp] = sc_pool.tile([P, N], f32, tag="mm",
                                          name=f"s{r}_{sub}_{hp}")
                # interleave even/odd head MMs -> concurrent row groups
                for qc in range(QC):
                    for hp in range(2):
                        hr = hp * HD
                        nc.tensor.matmul(
                            ss[hp][:, qc * FREE:(qc + 1) * FREE],
                            kt_sb[hi][hr:hr + HD, j * P:(j + 1) * P],
                            qt_sb[hi][hr:hr + HD, qc * FREE:(qc + 1) * FREE],
                            start=True, stop=True)
                # one attnV half after each sub: the round has TWO exp-wait
                # windows (sub1 waits exp(sub0); next round's sub0 waits
                # exp(sub1)), so the previous round's attnV is split into
                # per-head halves to fill both.
                if r > 0:
                    attn_half(*divmod(r - 1, JP), sub)
                    if sub == 1 and (r - 1) % JP == JP - 1:
                        hprev = (r - 1) // JP
                        rs2 = evac_raws(hprev)

                        def mk(hi=hprev, rs2=rs2, rnext=r + 2):
                            rc = pair_recips(hi, rs2)
                            finish_head(hi, 0, rc)
                            pending.setdefault(rnext, []).append(
                                lambda: finish_head(hi, 1, rc))
                        pending.setdefault(r + 1, []).append(mk)
                # exp: ACT always takes the even-head (se) slab -- it is the
                # slot the NEXT round's first scores MM reuses, and ACT's
                # 1.03us slab beats DVE's 1.24us, so the critical slot frees
                # first. DVE (Schraudolph) takes the odd-head slab.
                nc.scalar.activation(new_e2[0][:, sub, :], ss[0][:],
                                     Act.Exp, scale=SCALE)
                nc.vector.tensor_scalar(
                    out=new_e2[1][:, sub, :].bitcast(i8),
                    in0=ss[1][:], scalar1=A_EXP8,
                    scalar2=B_EXP8, op0=Alu.mult, op1=Alu.add)
            for hp in range(2):
                e2_of[2 * hi + hp] = new_e2[hp]
        for sub in range(2):
            attn_half(CT - 1, JP - 1, sub)
        rs2 = evac_raws(CT - 1)
        rc = pair_recips(CT - 1, rs2)
        for hp in range(2):
            finish_head(CT - 1, hp, rc)

    if dbg is not None:
        for i in range(CT):
            nc.sync.dma_start(out=dbg["aot"][i * P:(i + 1) * P, :], in_=aot_sb[i][:])

    # ---------------- stage D: out_proj + residual + LayerNorm ----------------
    # qc-major pipeline: each query half runs out-proj -> residual -> stats
    # -> LN chain -> apply -> DMA independently (LN is per-token), so qc0's
    # vector/scalar chain overlaps qc1's matmuls. out-proj iterates kt OUTER
    # so only the kt=3 matmuls (8 of 32) wait on the last head's normalize.
    with tc.tile_pool(name="stageD", bufs=1) as d_pool, \
         tc.tile_pool(name="tmpD", bufs=2) as tmpd_pool, \
         tc.tile_pool(name="psO", bufs=6, space="PSUM") as ps_o, \
         tc.tile_pool(name="psSt", bufs=2, space="PSUM") as ps_st:
        x_sb = [d_pool.tile([P, N], bf16, tag=f"x{i}", name=f"x{i}") for i in range(CT)]
        sq_sb = [d_pool.tile([P, N], bf16, tag=f"sq{i}", name=f"sq{i}") for i in range(CT)]

        # both query halves' out-proj matmuls first: the stats matmuls for a
        # half depend on its DVE residual/square chain, so running the other
        # half's out-proj in between keeps PE busy while DVE catches up.
        for qc in range(QC):
            sl = slice(qc * FREE, (qc + 1) * FREE)
            pos = [ps_o.tile([P, FREE], f32, tag="o", name=f"o{qc}_{ct}")
                   for ct in range(CT)]
            for kt in range(CT):
                for ct in range(CT):
                    nc.tensor.matmul(
                        pos[ct][:],
                        wo_sb[kt][:, ct * P:(ct + 1) * P],
                        aot_sb[kt][:, sl],
                        start=(kt == 0), stop=(kt == CT - 1))
            # x = proj + residual (bf16), then x^2
            for ct in range(CT):
                nc.vector.scalar_tensor_tensor(
                    out=x_sb[ct][:, sl], in0=pos[ct][:], scalar=1.0,
                    in1=xc_sb[ct][:, sl], op0=Alu.mult, op1=Alu.add)
                nc.vector.tensor_mul(sq_sb[ct][:, sl], x_sb[ct][:, sl],
                                     x_sb[ct][:, sl])

        for qc in range(QC):
            sl = slice(qc * FREE, (qc + 1) * FREE)
            # LN stats: s1 at partition 0, s2 at partition 32 of one PSUM
            # tile (separate col groups -> the ones-matmuls can overlap).
            s12 = ps_st.tile([33, FREE], f32, tag="s12", name=f"s12_{qc}")
            for ct in range(CT):
                nc.tensor.matmul(
                    s12[0:1, :], ones_col[:], x_sb[ct][:, sl],
                    start=(ct == 0), stop=(ct == CT - 1))
                nc.tensor.matmul(
                    s12[32:33, :], ones_col[:], sq_sb[ct][:, sl],
                    start=(ct == 0), stop=(ct == CT - 1))

            mu = d_pool.tile([1, FREE], f32, tag=f"mu{qc}")
            nc.vector.tensor_scalar_mul(mu[:], s12[0:1, :], 1.0 / C)
            mu2 = d_pool.tile([1, FREE], f32, tag=f"mu2{qc}")
            nc.vector.tensor_mul(mu2[:], mu[:], mu[:])
            var = d_pool.tile([1, FREE], f32, tag=f"var{qc}")
            nc.vector.scalar_tensor_tensor(
                out=var[:], in0=s12[32:33, :], scalar=1.0 / C, in1=mu2[:],
                op0=Alu.mult, op1=Alu.subtract)
            # rinv = 1/sqrt(var+eps) = exp(-0.5*ln(var+eps)); Ln/Exp share an
            # activation table set so no table switch (Sqrt would force one).
            lnv = d_pool.tile([1, FREE], f32, tag=f"lnv{qc}")
            nc.scalar.activation(lnv[:], var[:], Act.Ln, bias=eps_t[:])
            rinv = d_pool.tile([1, FREE], bf16, tag=f"rinv{qc}")
            nc.scalar.activation(rinv[:], lnv[:], Act.Exp, scale=-0.5)
            m2 = d_pool.tile([1, FREE], bf16, tag=f"m2{qc}")
            nc.vector.tensor_mul(m2[:], mu[:], rinv[:])
            ri_b = d_pool.tile([P, FREE], bf16, tag=f"rib{qc}")
            nc.gpsimd.partition_broadcast(ri_b[:], rinv[:])
            m2_b = d_pool.tile([P, FREE], bf16, tag=f"m2b{qc}")
            nc.gpsimd.partition_broadcast(m2_b[:], m2[:])

            # y = (x*rinv - mu*rinv) * gamma + beta
            for ct in range(CT):
                t = tmpd_pool.tile([P, FREE], bf16, tag="t")
                nc.vector.tensor_mul(t[:], x_sb[ct][:, sl], ri_b[:])
                t2 = tmpd_pool.tile([P, FREE], bf16, tag="t2")
                nc.vector.tensor_sub(t2[:], t[:], m2_b[:])
                y_sb = tmpd_pool.tile([P, FREE], f32, tag="y")
                nc.vector.tensor_scalar(
                    out=y_sb[:], in0=t2[:], scalar1=gamma_ct[ct][:],
                    scalar2=beta_ct[ct][:], op0=Alu.mult, op1=Alu.add)
                if ct % 2 == 0:
                    nc.sync.dma_start(out=y_d[ct * P:(ct + 1) * P, sl], in_=y_sb[:])
                else:
                    nc.scalar.dma_start(out=y_d[ct * P:(ct + 1) * P, sl], in_=y_sb[:])

            if dbg is not None:
                nc.sync.dma_start(out=dbg["st"][0:1, sl], in_=mu[:])
                nc.sync.dma_start(out=dbg["st"][1:2, sl], in_=var[:])
                st_tmp = d_pool.tile([1, FREE], f32, tag=f"sttmp{qc}")
                nc.vector.tensor_copy(out=st_tmp[:], in_=rinv[:])
                nc.sync.dma_start(out=dbg["st"][2:3, sl], in_=st_tmp[:])

        if dbg is not None:
            for i in range(CT):
                nc.sync.dma_start(out=dbg["x"][i * P:(i + 1) * P, :], in_=x_sb[i][:])


def _build(reps=1):
    from contextlib import ExitStack

    import concourse.tile as tile
    from concourse import bacc, mybir

    f32 = mybir.dt.float32
    bf16 = mybir.dt.bfloat16
    e4 = mybir.dt.float8e4
    nc = bacc.Bacc("TRN2", target_bir_lowering=False, debug=False,
                   num_devices=NUM_CORES)
    xc_d = nc.dram_tensor("xc", [C, N], bf16, kind="ExternalInput").ap()
    w_d = {
        "o": nc.dram_tensor("wot", [C, C], bf16, kind="ExternalInput").ap(),
        "k8": nc.dram_tensor("wk8", [2 * P, 2, C], e4, kind="ExternalInput").ap(),
        "v8": nc.dram_tensor("wv8", [2 * P, 2, C], e4, kind="ExternalInput").ap(),
        "q8": nc.dram_tensor("wq8", [2 * P, 2, C], e4, kind="ExternalInput").ap(),
        "xn8": nc.dram_tensor("xn8", [2 * P, 2, N2], e4, kind="ExternalInput").ap(),
        "xc8": nc.dram_tensor("xc8", [2 * P, 2, N], e4, kind="ExternalInput").ap(),
    }
    gamma_d = nc.dram_tensor("gamma", [C, 1], f32, kind="ExternalInput").ap()
    beta_d = nc.dram_tensor("beta", [C, 1], f32, kind="ExternalInput").ap()
    y_d = nc.dram_tensor("y", [C, N], f32, kind="ExternalOutput").ap()

    with tile.TileContext(nc) as tc:
        for _ in range(reps):
            with ExitStack() as ctx:
                _emit(ctx, tc, (xc_d, w_d, gamma_d, beta_d, y_d))
    nc.compile()
    return nc


def _get_nc(reps=1):
    key = ("nc", reps)
    if key not in _CACHE:
        _CACHE[key] = _build(reps)
    return _CACHE[key]


def _bf16(a):
    import ml_dtypes
    return np.asarray(a, dtype=np.float32).astype(ml_dtypes.bfloat16)


def _e4i(a):
    """[C, T] f32 -> [256, 2, T] e4m3 with c = pass*256 + ko*128 + ki
    mapped to (row = pass*128 + ki, ko) -- the DoubleRow interleave."""
    import ml_dtypes
    a = np.asarray(a, dtype=np.float32)
    T = a.shape[1]
    return np.ascontiguousarray(
        a.reshape(2, 2, P, T).transpose(0, 2, 1, 3).reshape(2 * P, 2, T)
    ).astype(ml_dtypes.float8_e4m3)


def make_in_maps(f_curr, f_prev, f_next, Wq, Wk, Wv, Wo, gamma, beta):
    f_curr = np.asarray(f_curr, dtype=np.float32).reshape(B, C, N)
    f_prev = np.asarray(f_prev, dtype=np.float32).reshape(B, C, N)
    f_next = np.asarray(f_next, dtype=np.float32).reshape(B, C, N)
    xn = np.concatenate([f_prev, f_next], axis=2)  # [B, C, 2N]
    shared = {
        "wq8": _e4i(np.asarray(Wq, dtype=np.float32).T),
        "wk8": _e4i(np.asarray(Wk, dtype=np.float32).T),
        "wv8": _e4i(np.asarray(Wv, dtype=np.float32).T),
        "wot": _bf16(np.ascontiguousarray(np.asarray(Wo, dtype=np.float32).T)),
        "gamma": np.asarray(gamma, dtype=np.float32).reshape(C, 1),
        "beta": np.asarray(beta, dtype=np.float32).reshape(C, 1),
    }
    return [
        {"xc": _bf16(f_curr[b]), "xn8": _e4i(xn[b]), "xc8": _e4i(f_curr[b]),
         **shared}
        for b in range(NUM_CORES)
    ]


def kernel(f_curr, f_prev, f_next, Wq, Wk, Wv, Wo, gamma, beta):
    from concourse.bass_utils import run_bass_kernel_spmd

    nc = _get_nc()
    in_maps = make_in_maps(f_curr, f_prev, f_next, Wq, Wk, Wv, Wo, gamma, beta)
    res = run_bass_kernel_spmd(nc, in_maps, list(range(NUM_CORES)))
    out = np.stack([res.results[b]["y"] for b in range(NUM_CORES)])
    return out.reshape(B, C, H, W).astype(np.float32)
